# revision 31
# baseline (speedup 1.0000x reference)
"""Trainium2 Bass kernel for nn_Attention_42331197669853 (PVT-style SR attention).

Math (B=2, N=4096, C=1024, H=16, HD=64, SR=2, M=2048):
  q  = (x @ Wq + bq)                     -> [B,H,N,64]
  kv = (LN(conv1d_SR(x; Wsr) + bsr)) * gamma + beta
  k,v = kv @ Wkv + bkv                   -> [B,H,M,64] each
  out = softmax(q k^T / 8) v             -> [B,N,C]
  y  = out @ Wproj + bproj

Sharding: 8 cores = (b in {0,1}) x (head-group g in {0..3}, 4 heads each).
Each core computes its 4 heads' attention for its batch; the final projection
is row-split over heads, partials are summed on the host (bias added there).

Per-core device pipeline (all matmuls in float32r):
  pass 1: stream x in n-chunks of 256 -> PE-transpose -> SR conv (full M,
          redundant within a b-group) -> LayerNorm -> transpose -> lnT,
          bounced to DRAM (SBUF can't hold Wsr + lnT + streams at once)
  phase B: reload lnT; build kT [2x64, 2, M] and V' [M, 4*65] (65th col =
          softmax-denominator ones; bottom ones-row folds biases; gamma/beta
          folded into Wkv host-side)
  pass 2: stream x in n-chunks of 512 -> transpose -> qT chunk -> per head:
          S^T = kT-slice @ qT (K=64), exp on ACT (scale=1/8),
          O' = V'.T @ E accumulated over m-tiles in PSUM ([65, n]: row 64 =
          softmax denominator), normalize via reciprocal + ones-matmul
          broadcast, then proj partial y = OnT.T @ Wproj rows -> DMA out.
"""

import os
import sys

for _p in ("/opt/trn_rl_repo",):
    if _p not in sys.path and os.path.isdir(_p):
        sys.path.append(_p)

import numpy as np

import concourse.bass as bass
import concourse.tile as tile
from concourse import mybir, bacc
from concourse.masks import make_identity

B, N, C, H, SR = 2, 4096, 1024, 16, 2
M = N // SR
HD = C // H
G = 4  # heads per core
EPS = 1e-3
SCALE = HD ** -0.5

F32 = mybir.dt.float32
F32R = mybir.dt.float32r

P = 128
CT = C // P  # 8 k-tiles over C
P1CH = 16  # pass-1 chunks (256 n each)
P2CH = 8  # pass-2 chunks (512 n each)
MT = M // P  # 16 m-tiles

# Collective mode: each core SR-convs only its m-quarter (x pre-rotated on the
# host so chunks 0-3 are its own quarter), then the 4 cores of a batch group
# AllGather the LN'd result. Deduplicates the SR conv 4x.
COLLECTIVE = True
REPLICA_GROUPS = [[0, 1, 2, 3], [4, 5, 6, 7]]
Exp = mybir.ActivationFunctionType.Exp
Identity = mybir.ActivationFunctionType.Identity
Sqrt = mybir.ActivationFunctionType.Sqrt


def build_nc():
    nc = bacc.Bacc("TRN2", target_bir_lowering=False, debug=False, num_devices=8)
    dt = lambda name, shape, out=False: nc.dram_tensor(
        name, shape, F32, kind="ExternalOutput" if out else "ExternalInput"
    ).ap()

    x_d = dt("x", [N, C])
    wq_d = dt("wq", [C, 2 * P])  # q cols for this group's 4 heads
    bq_d = dt("bq", [P, 2])
    wsr_d = dt("wsr", [SR, C, C])
    bsr_d = dt("bsr", [1, C])
    wk_d = dt("wk", [C, 2 * P])  # gamma-folded k cols
    bk_d = dt("bk", [P, 2])
    wv_d = dt("wv", [C, G * 65])  # gamma-folded v cols + zero 65th col per head
    wv1_d = dt("wv1", [1, G * 65])  # bias row + ones in 65th cols
    wpr_d = dt("wproj", [2 * P, C])  # proj rows for this group's heads
    y_d = dt("y", [N, C], out=True)

    x_r1 = x_d.rearrange("(ch nt p) c -> ch p nt c", p=P, nt=2)  # 16 chunks
    x_r2 = x_d.rearrange("(ch hf nt p) c -> ch hf p nt c", p=P, nt=2, hf=2)
    y_r2 = y_d.rearrange("(ch hf nt p) c -> ch hf p nt c", p=P, nt=2, hf=2)

    with tile.TileContext(nc) as tc:
        with tc.tile_pool(name="misc", bufs=1) as mp, tc.tile_pool(
            name="late", bufs=1
        ) as lp, tc.tile_pool(name="dram", bufs=1, space="DRAM") as dp:
            ident_f = mp.tile([P, P], F32)
            make_identity(nc, ident_f)
            ident = mp.tile([P, P], F32R)
            nc.vector.tensor_copy(ident[:], ident_f[:])
            ones_f = mp.tile([1, P], F32)
            nc.vector.memset(ones_f, 1.0)
            ones_r = mp.tile([1, P], F32R)
            nc.vector.tensor_copy(ones_r[:], ones_f[:])
            eps_t = mp.tile([P, 1], F32)
            nc.vector.memset(eps_t, EPS)
            bq_sb = mp.tile([P, 2], F32)
            nc.sync.dma_start(out=bq_sb[:], in_=bq_d)
            bk_sb = mp.tile([P, 2], F32)
            nc.sync.dma_start(out=bk_sb[:], in_=bk_d)
            bsr_f = mp.tile([1, C], F32)
            nc.sync.dma_start(out=bsr_f[:], in_=bsr_d)
            bsr_r = mp.tile([1, C], F32R)
            nc.vector.tensor_copy(bsr_r[:], bsr_f[:])

            # late-loaded tiles (space reserved now, DMA'd during/after pass 1)
            wq_r = lp.tile([P, CT, 2 * P], F32R)
            wk_r = lp.tile([P, CT, 2 * P], F32R)
            wv_r = lp.tile([P, CT, G * 65], F32R)
            wv1_r = lp.tile([1, G * 65], F32R)
            wpr_r = lp.tile([P, 2, C], F32R)
            kT = lp.tile([P, 2, M], F32R)  # [2x64 head pair, pair, m]
            vp = lp.tile([P, MT, G * 65], F32R)  # V' per m-tile, 65 cols/head

            qT_dram = dp.tile([P, 2, N], F32)
            if COLLECTIVE:
                lnq_dram = dp.tile([P, CT, M // 4], F32)
                lnT_gath = dp.tile([4, P, CT, M // 4], F32)
            else:
                lnT_dram = dp.tile([P, CT, M], F32)

            # ------------- pass 1: SR conv + LN -> lnT (to DRAM) -------------
            with tc.tile_pool(name="p_wsr", bufs=1) as pw, tc.tile_pool(
                name="st1", bufs=2
            ) as st1, tc.tile_pool(name="ps1", bufs=2, space="PSUM") as ps1:
                wsr_rr = wsr_d.rearrange("j (t p) c -> p j t c", p=P)
                wsr_r = pw.tile([P, SR, CT, C], F32R)
                for j in range(SR):
                    for t in range(CT):
                        wsst = st1.tile([P, C], F32, tag="stage")
                        nc.sync.dma_start(out=wsst[:], in_=wsr_rr[:, j, t, :])
                        nc.vector.tensor_copy(wsr_r[:, j, t, :], wsst[:])

                def load_rounded(dst, dram_ap):
                    # dst: [P, a, b] SBUF f32r slice; dram_ap same shape, fp32
                    a, b = dst.shape[1], dst.shape[2]
                    assert a * b <= 1056
                    stage = st1.tile([P, 1056], F32, tag="stage")
                    sv = stage[:, : a * b].rearrange("p (a b) -> p a b", b=b)
                    nc.sync.dma_start(out=sv, in_=dram_ap)
                    nc.vector.tensor_copy(dst, sv)

                wq_src = wq_d.rearrange("(t p) o -> p t o", p=P)
                wk_src = wk_d.rearrange("(t p) o -> p t o", p=P)
                wv_src = wv_d.rearrange("(t p) o -> p t o", p=P)
                wpr_src = wpr_d.rearrange("(t p) c -> p t c", p=P)
                for piece in range(2):
                    sl = slice(4 * piece, 4 * piece + 4)
                    load_rounded(wq_r[:, sl, :], wq_src[:, sl, :])
                    load_rounded(wk_r[:, sl, :], wk_src[:, sl, :])
                    load_rounded(wv_r[:, sl, :], wv_src[:, sl, :])
                    pr = slice(piece, piece + 1)
                    load_rounded(wpr_r[:, pr, :], wpr_src[:, pr, :])
                wv1_f = st1.tile([P, 1056], F32, tag="stage")
                nc.sync.dma_start(out=wv1_f[0:1, : G * 65], in_=wv1_d)
                nc.vector.tensor_copy(wv1_r[:, :], wv1_f[0:1, : G * 65])

                for p in range(P1CH):
                    xT = st1.tile([P, CT, 2 * P], F32R, tag="xT")
                    for nt in range(2):
                        xs = st1.tile([P, C], F32, tag="xs")
                        nc.sync.dma_start(out=xs[:], in_=x_r1[p, :, nt, :])
                        for ct in range(CT):
                            tp = ps1.tile([P, P], F32, tag="tp", bufs=4)
                            nc.tensor.transpose(
                                tp[:, :],
                                xs[:, ct * P : (ct + 1) * P],
                                ident_f[:, :],
                            )
                            # alternate copy engine: ACT is idle in pass 1
                            if ct % 2 == 0:
                                nc.scalar.copy(xT[:, ct, nt * P : (nt + 1) * P], tp)
                            else:
                                nc.vector.tensor_copy(
                                    xT[:, ct, nt * P : (nt + 1) * P], tp
                                )

                    # q projection for this chunk -> qT_dram
                    qch = st1.tile([P, 2, 2 * P], F32R, tag="qch")
                    for pair in range(2):
                        qps = ps1.tile([P, 2 * P], F32, tag="kvps")
                        for ct in range(CT):
                            nc.tensor.matmul(
                                qps[:, :],
                                wq_r[:, ct, pair * P : (pair + 1) * P],
                                xT[:, ct, :],
                                start=(ct == 0),
                                stop=(ct == CT - 1),
                            )
                        nc.scalar.activation(
                            out=qch[:, pair, :],
                            in_=qps[:, :],
                            func=Identity,
                            bias=bq_sb[:, pair : pair + 1],
                        )
                    nc.sync.dma_start(
                        out=qT_dram[:, :, p * 2 * P : (p + 1) * 2 * P],
                        in_=qch.bitcast(F32),
                    )

                    if COLLECTIVE and p >= 4:
                        continue  # SR conv only for own quarter (chunks 0-3)

                    xT_j = xT.rearrange("p t (m j) -> p t j m", j=SR)
                    kv_sb = st1.tile([P, C], F32, tag="kv")
                    for cc in range(2):
                        kvps = ps1.tile([P, 512], F32, tag="kvps")
                        first = True
                        for j in range(SR):
                            for ct in range(CT):
                                nc.tensor.matmul(
                                    kvps[:, :],
                                    xT_j[:, ct, j, :],
                                    wsr_r[:, j, ct, cc * 512 : (cc + 1) * 512],
                                    start=first,
                                    stop=False,
                                )
                                first = False
                        nc.tensor.matmul(
                            kvps[:, :],
                            ones_r[:, :],
                            bsr_r[:, cc * 512 : (cc + 1) * 512],
                            start=False,
                            stop=True,
                        )
                        nc.scalar.copy(kv_sb[:, cc * 512 : (cc + 1) * 512], kvps)

                    # LayerNorm over C
                    stats = st1.tile([P, 2, 6], F32, tag="st")
                    for sgi in range(2):
                        nc.vector.bn_stats(
                            out=stats[:, sgi, :],
                            in_=kv_sb[:, sgi * 512 : (sgi + 1) * 512],
                        )
                    mv = st1.tile([P, 2], F32, tag="mv")
                    nc.vector.bn_aggr(out=mv[:, :], in_=stats[:, :, :])
                    std = st1.tile([P, 1], F32, tag="sd")
                    nc.scalar.activation(
                        out=std[:, :], in_=mv[:, 1:2], func=Sqrt, bias=eps_t[:, 0:1]
                    )
                    rstd = st1.tile([P, 1], F32, tag="rs")
                    nc.vector.reciprocal(rstd[:, :], std[:, :])
                    ln_r = st1.tile([P, C], F32R, tag="ln")
                    nc.vector.tensor_scalar(
                        out=ln_r[:, :],
                        in0=kv_sb[:, :],
                        scalar1=mv[:, 0:1],
                        scalar2=rstd[:, 0:1],
                        op0=mybir.AluOpType.subtract,
                        op1=mybir.AluOpType.mult,
                    )
                    lnch = st1.tile([P, CT, P], F32R, tag="lnch")
                    for ct in range(CT):
                        tp2 = ps1.tile([P, P], F32R, tag="tp2")
                        nc.tensor.transpose(
                            tp2[:, :], ln_r[:, ct * P : (ct + 1) * P], ident[:, :]
                        )
                        nc.vector.tensor_copy(lnch[:, ct, :], tp2)
                    ln_dst = lnq_dram if COLLECTIVE else lnT_dram
                    nc.sync.dma_start(
                        out=ln_dst[:, :, p * P : (p + 1) * P], in_=lnch.bitcast(F32)
                    )

                if COLLECTIVE:
                    nc.gpsimd.collective_compute(
                        "AllGather",
                        mybir.AluOpType.bypass,
                        replica_groups=REPLICA_GROUPS,
                        ins=[lnq_dram.opt()],
                        outs=[lnT_gath.opt()],
                    )

            # ------------- phase B: reload lnT, build kT and V' -------------
            with tc.tile_pool(name="p_lnT", bufs=1) as pl, tc.tile_pool(
                name="psB", bufs=2, space="PSUM"
            ) as psB:
                lnT = pl.tile([P, CT, M], F32R)
                for qu in range(4):
                    msl = slice(qu * 512, (qu + 1) * 512)
                    src = lnT_gath[qu] if COLLECTIVE else lnT_dram[:, :, msl]
                    nc.sync.dma_start(out=lnT[:, :, msl].bitcast(F32), in_=src)
                    # re-round in place so the verifier sees an F32R producer
                    nc.vector.tensor_copy(lnT[:, :, msl], lnT[:, :, msl].bitcast(F32))
                lnTf = lnT

                for pair in range(2):
                    for mch in range(4):
                        kps = psB.tile([P, 512], F32, tag="k")
                        for ct in range(CT):
                            nc.tensor.matmul(
                                kps[:, :],
                                wk_r[:, ct, pair * P : (pair + 1) * P],
                                lnTf[:, ct, mch * 512 : (mch + 1) * 512],
                                start=(ct == 0),
                                stop=(ct == CT - 1),
                            )
                        nc.scalar.activation(
                            out=kT[:, pair, mch * 512 : (mch + 1) * 512],
                            in_=kps[:, :],
                            func=Identity,
                            bias=bk_sb[:, pair : pair + 1],
                        )
                for mt in range(MT):
                    vps = psB.tile([P, G * 65], F32, tag="v")
                    for ct in range(CT):
                        nc.tensor.matmul(
                            vps[:, :],
                            lnT[:, ct, mt * P : (mt + 1) * P],
                            wv_r[:, ct, :],
                            start=(ct == 0),
                            stop=False,
                        )
                    nc.tensor.matmul(
                        vps[:, :], ones_r[:, :], wv1_r[:, :], start=False, stop=True
                    )
                    nc.vector.tensor_copy(vp[:, mt, :], vps[:, :])

            # ------------- pass 2: q, attention, proj -------------
            EW = 2  # m-tiles per exp instruction
            with tc.tile_pool(name="st2", bufs=2) as st2, tc.tile_pool(
                name="psS", bufs=3, space="PSUM"
            ) as psS, tc.tile_pool(name="psA", bufs=2, space="PSUM") as psA:
                for ch in range(P2CH):
                    qTc = st2.tile([P, 2, 512], F32R, tag="qTc")
                    nc.sync.dma_start(
                        out=qTc.bitcast(F32),
                        in_=qT_dram[:, :, ch * 512 : (ch + 1) * 512],
                    )
                    nc.vector.tensor_copy(qTc[:, :, :], qTc[:, :, :].bitcast(F32))

                    onT = st2.tile([P, 2, 512], F32R, tag="onT")
                    for h in range(G):
                        pr, po = h // 2, 64 * (h % 2)
                        ops = psA.tile([65, 512], F32, tag="acc")
                        mt0 = 0
                        while mt0 < MT:
                            w = min(EW, MT - mt0)
                            sps = psS.tile([P, EW, 512], F32, tag="s")
                            for i in range(w):
                                mt = mt0 + i
                                nc.tensor.matmul(
                                    sps[:, i, :],
                                    kT[po : po + 64, pr, mt * P : (mt + 1) * P],
                                    qTc[po : po + 64, pr, :],
                                    start=True,
                                    stop=True,
                                )
                            e_t = st2.tile([P, EW, 512], F32R, tag="e")
                            nc.scalar.activation(
                                out=e_t[:, :w, :], in_=sps[:, :w, :], func=Exp,
                                scale=SCALE,
                            )
                            for i in range(w):
                                mt = mt0 + i
                                nc.tensor.matmul(
                                    ops[:, :],
                                    vp[:, mt, h * 65 : (h + 1) * 65],
                                    e_t[:, i, :],
                                    start=(mt == 0),
                                    stop=(mt == MT - 1),
                                )
                            mt0 += w
                        rc = st2.tile([1, 512], F32, tag="rc")
                        nc.vector.reciprocal(rc[:, :], ops[64:65, :])
                        bc_sb = st2.tile([64, 512], F32, tag="bcs")
                        nc.gpsimd.partition_broadcast(bc_sb[:, :], rc[:, :])
                        nc.vector.tensor_mul(
                            onT[po : po + 64, pr, :], ops[0:64, :], bc_sb[:, :]
                        )

                    for hf in range(2):
                        y_sb = st2.tile([P, 2, C], F32, tag="ysb")
                        for nt in range(2):
                            yps = psS.tile([P, 2, 512], F32, tag="s")
                            for cc in range(2):
                                for pair in range(2):
                                    nc.tensor.matmul(
                                        yps[:, cc, :],
                                        onT[:, pair, (2 * hf + nt) * P : (2 * hf + nt + 1) * P],
                                        wpr_r[:, pair, cc * 512 : (cc + 1) * 512],
                                        start=(pair == 0),
                                        stop=(pair == 1),
                                    )
                            nc.scalar.copy(y_sb[:, nt, :], yps.rearrange("p a b -> p (a b)"))
                        nc.sync.dma_start(out=y_r2[ch, hf], in_=y_sb[:])

    nc.compile()
    return nc


_NC_CACHE = None


def _get_nc():
    global _NC_CACHE
    if _NC_CACHE is None:
        _NC_CACHE = build_nc()
    return _NC_CACHE


def _host_prep(inputs):
    """Build the 8 per-core input maps."""
    x = np.asarray(inputs["x"], np.float32)
    Wq = np.asarray(inputs["Wq"], np.float32)
    bq = np.asarray(inputs["bq"], np.float32)
    Wsr = np.asarray(inputs["Wsr"], np.float32)
    bsr = np.asarray(inputs["bsr"], np.float32)
    gamma = np.asarray(inputs["gamma"], np.float32)
    beta = np.asarray(inputs["beta"], np.float32)
    Wkv = np.asarray(inputs["Wkv"], np.float32)
    bkv = np.asarray(inputs["bkv"], np.float32)
    Wproj = np.asarray(inputs["Wproj"], np.float32)

    Wkv_eff = gamma[:, None] * Wkv
    bkv_eff = beta @ Wkv + bkv  # [2C]

    in_maps = []
    for core in range(8):
        b, g = divmod(core, 4)
        if COLLECTIVE:
            perm = _perm(g)
            x_b = np.ascontiguousarray(
                x[b].reshape(P1CH, N // P1CH, C)[perm].reshape(N, C)
            )
        else:
            x_b = np.ascontiguousarray(x[b])
        cs = slice(256 * g, 256 * (g + 1))
        wv_cols = Wkv_eff[:, C + 256 * g : C + 256 * (g + 1)]  # [C, 256]
        bv = bkv_eff[C + 256 * g : C + 256 * (g + 1)]  # [256]
        wv_aug = np.zeros((C, G * 65), np.float32)
        wv1 = np.zeros((1, G * 65), np.float32)
        for h in range(G):
            wv_aug[:, h * 65 : h * 65 + 64] = wv_cols[:, h * 64 : (h + 1) * 64]
            wv1[0, h * 65 : h * 65 + 64] = bv[h * 64 : (h + 1) * 64]
            wv1[0, h * 65 + 64] = 1.0
        in_maps.append(
            {
                "x": x_b,
                "wq": np.ascontiguousarray(Wq[:, cs]),
                "bq": np.ascontiguousarray(bq[cs].reshape(2, P).T),
                "wsr": Wsr,
                "bsr": bsr.reshape(1, C),
                "wk": np.ascontiguousarray(Wkv_eff[:, cs]),
                "bk": np.ascontiguousarray(bkv_eff[cs].reshape(2, P).T),
                "wv": wv_aug,
                "wv1": wv1,
                "wproj": np.ascontiguousarray(Wproj[cs, :]),
            }
        )
    return in_maps


def _perm(g):
    """Pass-1 chunk order for head-group g: own m-quarter first."""
    return list(range(4 * g, 4 * g + 4)) + [c for c in range(P1CH) if c // 4 != g]


def kernel(**inputs) -> np.ndarray:
    from concourse.bass_utils import run_bass_kernel_spmd

    nc = _get_nc()
    in_maps = _host_prep(inputs)
    res = run_bass_kernel_spmd(nc, in_maps, core_ids=list(range(8)))
    bproj = np.asarray(inputs["bproj"], np.float32)
    y = np.zeros((B, N, C), np.float32)
    for core in range(8):
        b, g = divmod(core, 4)
        yp = res.results[core]["y"]
        if COLLECTIVE:
            yg = np.empty_like(yp)
            yg.reshape(P1CH, N // P1CH, C)[_perm(g)] = yp.reshape(
                P1CH, N // P1CH, C
            )
            yp = yg
        y[b] += yp
    y += bproj
    return y
